# revision 11
# baseline (speedup 1.0000x reference)
"""MinGRU (parallel log-space scan) Trainium2 Bass kernel.

Problem (hardcoded):
    x:    [B=8, S=4096, D=1024] f32
    W_hg: [D=1024, 2*D=2048]    f32
    out:  [B=8, S=4096, D=1024] f32

    hg = x @ W_hg ; hidden, gate = split(hg)
    h_t = (1-z_t) * h_{t-1} + z_t * g(hidden_t),  z = sigmoid(gate),
    g(v) = v + 0.5 if v >= 0 else sigmoid(v)  ==  max(v + 0.5, sigmoid(v))

Sharding: data-parallel over batch, one batch row per NeuronCore (8 cores),
W_hg replicated.

The scan runs along the free dimension (channels on partitions), so the
device works entirely in the transposed layout hg^T/h^T = [channels, seq];
the host passes x pre-transposed per batch row and transposes the returned
h^T back.

v2 layout/perf notes:
  - All matmul operands are bf16 (host-cast): same 1 cyc/row PE rate as
    fp32r but half the DMA bytes and fast weight loads (FWL).
  - x^T and W are fully resident in SBUF (96 KiB/partition at bf16), so
    there are no input DMAs or pool-recycle waits in steady state.
  - DMA trigger count is minimized (one trigger per x chunk / W k-slice):
    one HWDGE trigger spreads across all 16 SDMA engines, and the serial
    ~0.6us per-trigger issue cost on the Sync queue was the reason the
    baseline's first matmul started at 17us.
  - Input triggers are split across the two HWDGE rings: w[k=0] + x chunks
    on the Scalar ring, w[k=1..7] on the Sync ring, so the first matmul
    only waits for ~1 MiB. Output stores go on the Sync ring.
  - W is host-shuffled to [d, (k, half, c)] so each k-slice (hidden+gate
    128-col blocks for all 8 j) is one contiguous-column trigger.
  - Chunks are [256, 512*7, 256]: small first chunk starts the PE sooner,
    small last chunk (plus 128-col scan/store splits for the last two k)
    shortens the serial pointwise tail after the final matmul.

Per-core pipeline per (chunk sc, channel tile k):
  fp32-PSUM bf16 matmuls hg^T[k] = sum_j W[j,k]^T x^T[j]
  -> ACT: a = sigmoid(-gate), sigh = sigmoid(hidden)      [PSUM -> SBUF]
  -> DVE: gh = (hidden + 0.5) max sigh ; bneg = (a - 1) * gh
  -> DVE: h = scan(a * h_prev) - bneg   (carry chained across chunks)
  -> DMA h^T tile straight to DRAM out^T.
"""

import numpy as np
import ml_dtypes

import concourse.bacc as bacc
import concourse.tile as tile
from concourse import mybir

B, S, D = 8, 4096, 1024
N_CORES = 8
P = 128  # partitions
N_DT = D // P  # 8 contraction (j) tiles
N_KT = D // P  # 8 output channel (k) tiles per half
CHUNKS = [256] + [512] * 7 + [256]
assert sum(CHUNKS) == S

BF16 = mybir.dt.bfloat16
F32 = mybir.dt.float32

_COMPILED = {}


def _build():
    nc = bacc.Bacc(
        "TRN2", target_bir_lowering=False, debug=False, num_devices=N_CORES
    )
    # Both inputs are host-shuffled into the exact SBUF layout so every load
    # is a 2D copy of 128 DRAM rows with multi-KB contiguous lines: HWDGE
    # descriptor generation costs ~5ns per row, so tall/skinny APs (1024
    # rows) would serialize the preload and starve the PE.
    #   xt[p, 8*s0 + j*C + c] = x[s0+c, j*128+p]   (chunk-major)
    #   w[p, ((k*2+half)*8+j)*128+c] = W_hg[j*128+p, half*D+k*128+c]
    xt_d = nc.dram_tensor("xt", [P, N_DT * S], BF16, kind="ExternalInput").ap()
    w_d = nc.dram_tensor("w", [P, N_DT * 2 * D], BF16, kind="ExternalInput").ap()
    out_d = nc.dram_tensor("outT", [D, S], F32, kind="ExternalOutput").ap()

    AL = mybir.AluOpType
    SIG = mybir.ActivationFunctionType.Sigmoid

    with tile.TileContext(nc) as tc:
        with (
            tc.tile_pool(name="wpool", bufs=1) as wpool,
            tc.tile_pool(name="xpool", bufs=1) as x_pool,
            tc.tile_pool(name="pw", bufs=3) as pw_pool,
            tc.tile_pool(name="hp", bufs=3) as h_pool,
            tc.tile_pool(name="pshg", bufs=8, space="PSUM") as psum_hg,
        ):
            # SBUF weight tile, free layout (k, half, j, c): 16384 cols bf16
            wt = wpool.tile([P, N_DT * 2 * D], BF16, tag="w", name="wt")
            KB = 2 * N_DT * P  # 2048 cols per k

            def wload(eng, k):
                # one trigger: all 16 [128,128] blocks of output tile k
                # (contiguous 2048 cols / 4KB lines in the shuffled layout)
                eng.dma_start(
                    wt[:, k * KB : (k + 1) * KB], w_d[:, k * KB : (k + 1) * KB]
                )

            # x^T fully resident: one SBUF tile per chunk, free layout (j, c)
            xtiles = []
            starts = []
            s0 = 0
            for sc, C in enumerate(CHUNKS):
                t = x_pool.tile([P, N_DT * C], BF16, tag=f"xc{sc}", name=f"xc{sc}")
                xtiles.append(t)
                starts.append(s0)
                s0 += C

            def xload(sc):
                C, s0 = CHUNKS[sc], starts[sc]
                nc.scalar.dma_start(
                    xtiles[sc][:], xt_d[:, N_DT * s0 : N_DT * (s0 + C)]
                )

            # Concurrent transfers share SDMA bandwidth round-robin, so keep
            # the two startup-critical transfers (xc0, w[k=0]) FIRST on their
            # own rings: x chunks on the Scalar ring, W (then the output
            # stores) on the Sync ring.
            xload(0)
            for k in range(N_KT):
                wload(nc.sync, k)
            for sc in range(1, len(CHUNKS)):
                xload(sc)

            def w_sb(j, k, half):
                off = ((k * 2 + half) * N_DT + j) * P
                return wt[:, off : off + P]

            prev_h = [None] * N_KT
            last = len(CHUNKS) - 1
            for sc, C in enumerate(CHUNKS):
                s0 = starts[sc]
                xt_sb = xtiles[sc]
                for k in range(N_KT):
                    # Serial chain after the very last matmul: gate PSUM ->
                    # a -> bneg -> scan -> store. For the final k of the
                    # last chunk, run it in 128-col pieces (even the gate
                    # matmul group) so the pieces pipeline.
                    fin = sc == last and k == N_KT - 1
                    ph = psum_hg.tile([P, C], F32, tag="ph")  # hidden
                    for j in range(N_DT):
                        nc.tensor.matmul(
                            ph[:],
                            w_sb(j, k, 0),
                            xt_sb[:, j * C : (j + 1) * C],
                            start=(j == 0),
                            stop=(j == N_DT - 1),
                        )
                    pg = psum_hg.tile([P, C], F32, tag="ph")  # gate
                    gate_splits = (
                        [(0, C)]
                        if not fin
                        else [(c0, c0 + P) for c0 in range(0, C, P)]
                    )
                    for c0, c1 in gate_splits:
                        for j in range(N_DT):
                            nc.tensor.matmul(
                                pg[:, c0:c1],
                                w_sb(j, k, 1),
                                xt_sb[:, j * C + c0 : j * C + c1],
                                start=(j == 0),
                                stop=(j == N_DT - 1),
                            )
                    # sigh = sigmoid(hidden);  g = max(hidden+0.5, sigh)
                    sigh = pw_pool.tile([P, C], F32, tag="sigh")
                    nc.scalar.activation(sigh[:], ph[:], SIG)
                    gh = pw_pool.tile([P, C], F32, tag="gh")
                    nc.vector.scalar_tensor_tensor(
                        gh[:], ph[:], 0.5, sigh[:], op0=AL.add, op1=AL.max
                    )
                    a_t = pw_pool.tile([P, C], F32, tag="a")
                    bneg = pw_pool.tile([P, C], F32, tag="bneg")
                    h = h_pool.tile([P, C], F32, tag=f"h{k}")
                    carry = None if prev_h[k] is None else prev_h[k][:, -1:]
                    for c0, c1 in gate_splits:
                        # a = sigmoid(-gate) = 1 - z
                        nc.scalar.activation(
                            a_t[:, c0:c1], pg[:, c0:c1], SIG, scale=-1.0
                        )
                        # bneg = (a - 1) * g = -(z * g)
                        nc.vector.scalar_tensor_tensor(
                            bneg[:, c0:c1], a_t[:, c0:c1], 1.0, gh[:, c0:c1],
                            op0=AL.subtract, op1=AL.mult,
                        )
                        # h_t = a_t * h_{t-1} - bneg_t  (linear recurrence)
                        init = 0.0 if carry is None else carry
                        nc.vector.tensor_tensor_scan(
                            h[:, c0:c1], a_t[:, c0:c1], bneg[:, c0:c1], init,
                            op0=AL.mult, op1=AL.subtract,
                        )
                        carry = h[:, c1 - 1 : c1]
                        nc.sync.dma_start(
                            out_d[k * P : (k + 1) * P, s0 + c0 : s0 + c1],
                            h[:, c0:c1],
                        )
                    prev_h[k] = h
    nc.compile()
    return nc


def _get_nc():
    if "nc" not in _COMPILED:
        _COMPILED["nc"] = _build()
    return _COMPILED["nc"]


def make_in_maps(x: np.ndarray, W_hg: np.ndarray):
    bf = ml_dtypes.bfloat16
    # W_hg [j*128+p, half*D+k*128+c] -> w[p, ((k*2+half)*8+j)*128+c]
    w = np.asarray(W_hg, dtype=np.float32).reshape(N_DT, P, 2, N_KT, P)
    w = w.transpose(1, 3, 2, 0, 4).reshape(P, N_DT * 2 * D)
    w = np.ascontiguousarray(w).astype(bf)
    x = np.asarray(x, dtype=np.float32)
    in_maps = []
    for b in range(N_CORES):
        xb = x[b].astype(bf)  # [S, D]
        blocks = []
        s0 = 0
        for C in CHUNKS:
            # x[s0+c, j*128+p] -> [p, j*C+c]
            blk = xb[s0 : s0 + C].T.reshape(N_DT, P, C)
            blocks.append(blk.transpose(1, 0, 2).reshape(P, N_DT * C))
            s0 += C
        xt = np.ascontiguousarray(np.concatenate(blocks, axis=1))
        in_maps.append({"xt": xt, "w": w})
    return in_maps


def kernel(x: np.ndarray, W_hg: np.ndarray) -> np.ndarray:
    from concourse.bass_utils import run_bass_kernel_spmd

    assert x.shape == (B, S, D) and W_hg.shape == (D, 2 * D)
    nc = _get_nc()
    in_maps = make_in_maps(x, W_hg)
    res = run_bass_kernel_spmd(nc, in_maps, list(range(N_CORES)))
    out = np.empty((B, S, D), dtype=np.float32)
    for b in range(N_CORES):
        out[b] = res.results[b]["outT"].T
    return out


# revision 14
# speedup vs baseline: 1.0531x; 1.0531x over previous
"""MinGRU (parallel log-space scan) Trainium2 Bass kernel.

Problem (hardcoded):
    x:    [B=8, S=4096, D=1024] f32
    W_hg: [D=1024, 2*D=2048]    f32
    out:  [B=8, S=4096, D=1024] f32

    hg = x @ W_hg ; hidden, gate = split(hg)
    h_t = (1-z_t) * h_{t-1} + z_t * g(hidden_t),  z = sigmoid(gate),
    g(v) = v + 0.5 if v >= 0 else sigmoid(v)  ==  max(v + 0.5, sigmoid(v))

Sharding: data-parallel over batch, one batch row per NeuronCore (8 cores),
W_hg replicated.

The scan runs along the free dimension (channels on partitions), so the
device works entirely in the transposed layout hg^T/h^T = [channels, seq];
the host passes x pre-transposed per batch row and transposes the returned
h^T back.

v2 layout/perf notes:
  - All matmul operands are bf16 (host-cast): same 1 cyc/row PE rate as
    fp32r but half the DMA bytes and fast weight loads (FWL).
  - x^T and W are fully resident in SBUF (96 KiB/partition at bf16), so
    there are no input DMAs or pool-recycle waits in steady state.
  - DMA trigger count is minimized (one trigger per x chunk / W k-slice):
    one HWDGE trigger spreads across all 16 SDMA engines, and the serial
    ~0.6us per-trigger issue cost on the Sync queue was the reason the
    baseline's first matmul started at 17us.
  - Input triggers are split across the two HWDGE rings: w[k=0] + x chunks
    on the Scalar ring, w[k=1..7] on the Sync ring, so the first matmul
    only waits for ~1 MiB. Output stores go on the Sync ring.
  - W is host-shuffled to [d, (k, half, c)] so each k-slice (hidden+gate
    128-col blocks for all 8 j) is one contiguous-column trigger.
  - Chunks are [256, 512*7, 256]: small first chunk starts the PE sooner,
    small last chunk (plus 128-col scan/store splits for the last two k)
    shortens the serial pointwise tail after the final matmul.

Per-core pipeline per (chunk sc, channel tile k):
  fp32-PSUM bf16 matmuls hg^T[k] = sum_j W[j,k]^T x^T[j]
  -> ACT: a = sigmoid(-gate), sigh = sigmoid(hidden)      [PSUM -> SBUF]
  -> DVE: gh = (hidden + 0.5) max sigh ; bneg = (a - 1) * gh
  -> DVE: h = scan(a * h_prev) - bneg   (carry chained across chunks)
  -> DMA h^T tile straight to DRAM out^T.
"""

import numpy as np
import ml_dtypes

import concourse.bacc as bacc
import concourse.tile as tile
from concourse import mybir

B, S, D = 8, 4096, 1024
N_CORES = 8
P = 128  # partitions
N_DT = D // P  # 8 contraction (j) tiles
N_KT = D // P  # 8 output channel (k) tiles per half
CHUNKS = [256] + [512] * 7 + [256]
assert sum(CHUNKS) == S

BF16 = mybir.dt.bfloat16
F32 = mybir.dt.float32

_COMPILED = {}


def _build():
    nc = bacc.Bacc(
        "TRN2", target_bir_lowering=False, debug=False, num_devices=N_CORES
    )
    # Both inputs are host-shuffled into the exact SBUF layout so every load
    # is a 2D copy of 128 DRAM rows with multi-KB contiguous lines: HWDGE
    # descriptor generation costs ~5ns per row, so tall/skinny APs (1024
    # rows) would serialize the preload and starve the PE.
    #   xt[p, 8*s0 + j*C + c] = x[s0+c, j*128+p]   (chunk-major)
    #   w[p, ((k*2+half)*8+j)*128+c] = W_hg[j*128+p, half*D+k*128+c]
    xt_d = nc.dram_tensor("xt", [P, N_DT * S], BF16, kind="ExternalInput").ap()
    w_d = nc.dram_tensor("w", [P, N_DT * 2 * D], BF16, kind="ExternalInput").ap()
    out_d = nc.dram_tensor("outT", [D, S], F32, kind="ExternalOutput").ap()

    AL = mybir.AluOpType
    SIG = mybir.ActivationFunctionType.Sigmoid

    with tile.TileContext(nc) as tc:
        with (
            tc.tile_pool(name="wpool", bufs=1) as wpool,
            tc.tile_pool(name="xpool", bufs=1) as x_pool,
            tc.tile_pool(name="pw", bufs=3) as pw_pool,
            tc.tile_pool(name="hp", bufs=3) as h_pool,
            tc.tile_pool(name="pshg", bufs=8, space="PSUM") as psum_hg,
        ):
            # SBUF weight tile, free layout (k, half, j, c): 16384 cols bf16
            wt = wpool.tile([P, N_DT * 2 * D], BF16, tag="w", name="wt")
            KB = 2 * N_DT * P  # 2048 cols per k

            def wload(eng, c0, c1):
                # contiguous cols of the shuffled layout (multi-KB lines)
                eng.dma_start(wt[:, c0:c1], w_d[:, c0:c1])

            # x^T fully resident: one SBUF tile per chunk, free layout (j, c)
            xtiles = []
            starts = []
            s0 = 0
            for sc, C in enumerate(CHUNKS):
                t = x_pool.tile([P, N_DT * C], BF16, tag=f"xc{sc}", name=f"xc{sc}")
                xtiles.append(t)
                starts.append(s0)
                s0 += C

            def xload(sc):
                C, s0 = CHUNKS[sc], starts[sc]
                nc.scalar.dma_start(
                    xtiles[sc][:], xt_d[:, N_DT * s0 : N_DT * (s0 + C)]
                )

            # Startup choreography. Constraints: (a) concurrent transfers
            # share SDMA bandwidth round-robin, (b) the tile framework
            # tracks DMA completion on only 8 semaphore lanes shared by BOTH
            # HWDGE rings — a trigger that reuses a lane blocks its whole
            # ring FIFO until the lane's previous transfer completes. So:
            # keep at most ~6 transfers in flight at the start, put the two
            # critical pieces (xc0, w[k=0,hidden]) first on their own rings
            # (x on Scalar; W then stores on Sync), batch the rest of W into
            # deadline-ordered triggers, and stream the remaining x chunks
            # from inside the loop.
            xload(0)
            wload(nc.sync, 0, N_DT * P)  # k0 hidden
            wload(nc.sync, N_DT * P, KB)  # k0 gate
            xload(1)
            wload(nc.sync, 1 * KB, 3 * KB)  # k1-2
            wload(nc.sync, 3 * KB, 5 * KB)  # k3-4
            wload(nc.sync, 5 * KB, 8 * KB)  # k5-7
            xload(2)

            def w_sb(j, k, half):
                off = ((k * 2 + half) * N_DT + j) * P
                return wt[:, off : off + P]

            prev_h = [None] * N_KT
            last = len(CHUNKS) - 1
            for sc, C in enumerate(CHUNKS):
                s0 = starts[sc]
                xt_sb = xtiles[sc]
                for k in range(N_KT):
                    if k == 1 and sc + 3 < len(CHUNKS):
                        # issue the next x-chunk trigger from inside the
                        # loop (paced by the Scalar queue) so at most a few
                        # transfers are ever in flight
                        xload(sc + 3)
                    # Serial chain after the very last matmul: gate PSUM ->
                    # a -> bneg -> scan -> store. For the final k of the
                    # last chunk, run it in 128-col pieces (even the gate
                    # matmul group) so the pieces pipeline.
                    fin = sc == last and k == N_KT - 1
                    ph = psum_hg.tile([P, C], F32, tag="ph")  # hidden
                    for j in range(N_DT):
                        nc.tensor.matmul(
                            ph[:],
                            w_sb(j, k, 0),
                            xt_sb[:, j * C : (j + 1) * C],
                            start=(j == 0),
                            stop=(j == N_DT - 1),
                        )
                    pg = psum_hg.tile([P, C], F32, tag="ph")  # gate
                    gate_splits = (
                        [(0, C)]
                        if not fin
                        else [(c0, c0 + P) for c0 in range(0, C, P)]
                    )
                    for c0, c1 in gate_splits:
                        for j in range(N_DT):
                            nc.tensor.matmul(
                                pg[:, c0:c1],
                                w_sb(j, k, 1),
                                xt_sb[:, j * C + c0 : j * C + c1],
                                start=(j == 0),
                                stop=(j == N_DT - 1),
                            )
                    # sigh = sigmoid(hidden);  g = max(hidden+0.5, sigh)
                    sigh = pw_pool.tile([P, C], F32, tag="sigh")
                    nc.scalar.activation(sigh[:], ph[:], SIG)
                    gh = pw_pool.tile([P, C], F32, tag="gh")
                    nc.vector.scalar_tensor_tensor(
                        gh[:], ph[:], 0.5, sigh[:], op0=AL.add, op1=AL.max
                    )
                    a_t = pw_pool.tile([P, C], F32, tag="a")
                    bneg = pw_pool.tile([P, C], F32, tag="bneg")
                    h = h_pool.tile([P, C], F32, tag=f"h{k}")
                    carry = None if prev_h[k] is None else prev_h[k][:, -1:]
                    for c0, c1 in gate_splits:
                        # a = sigmoid(-gate) = 1 - z
                        nc.scalar.activation(
                            a_t[:, c0:c1], pg[:, c0:c1], SIG, scale=-1.0
                        )
                        # bneg = (a - 1) * g = -(z * g)
                        nc.vector.scalar_tensor_tensor(
                            bneg[:, c0:c1], a_t[:, c0:c1], 1.0, gh[:, c0:c1],
                            op0=AL.subtract, op1=AL.mult,
                        )
                        # h_t = a_t * h_{t-1} - bneg_t  (linear recurrence)
                        init = 0.0 if carry is None else carry
                        nc.vector.tensor_tensor_scan(
                            h[:, c0:c1], a_t[:, c0:c1], bneg[:, c0:c1], init,
                            op0=AL.mult, op1=AL.subtract,
                        )
                        carry = h[:, c1 - 1 : c1]
                        nc.sync.dma_start(
                            out_d[k * P : (k + 1) * P, s0 + c0 : s0 + c1],
                            h[:, c0:c1],
                        )
                    prev_h[k] = h
    nc.compile()
    return nc


def _get_nc():
    if "nc" not in _COMPILED:
        _COMPILED["nc"] = _build()
    return _COMPILED["nc"]


def make_in_maps(x: np.ndarray, W_hg: np.ndarray):
    bf = ml_dtypes.bfloat16
    # W_hg [j*128+p, half*D+k*128+c] -> w[p, ((k*2+half)*8+j)*128+c]
    w = np.asarray(W_hg, dtype=np.float32).reshape(N_DT, P, 2, N_KT, P)
    w = w.transpose(1, 3, 2, 0, 4).reshape(P, N_DT * 2 * D)
    w = np.ascontiguousarray(w).astype(bf)
    x = np.asarray(x, dtype=np.float32)
    in_maps = []
    for b in range(N_CORES):
        xb = x[b].astype(bf)  # [S, D]
        blocks = []
        s0 = 0
        for C in CHUNKS:
            # x[s0+c, j*128+p] -> [p, j*C+c]
            blk = xb[s0 : s0 + C].T.reshape(N_DT, P, C)
            blocks.append(blk.transpose(1, 0, 2).reshape(P, N_DT * C))
            s0 += C
        xt = np.ascontiguousarray(np.concatenate(blocks, axis=1))
        in_maps.append({"xt": xt, "w": w})
    return in_maps


def kernel(x: np.ndarray, W_hg: np.ndarray) -> np.ndarray:
    from concourse.bass_utils import run_bass_kernel_spmd

    assert x.shape == (B, S, D) and W_hg.shape == (D, 2 * D)
    nc = _get_nc()
    in_maps = make_in_maps(x, W_hg)
    res = run_bass_kernel_spmd(nc, in_maps, list(range(N_CORES)))
    out = np.empty((B, S, D), dtype=np.float32)
    for b in range(N_CORES):
        out[b] = res.results[b]["outT"].T
    return out
